# revision 23
# baseline (speedup 1.0000x reference)
"""Trainium2 Bass kernel for batched windowed multi-head attention.

Shapes: x (8, 64, 256, 512) f32, H=8 heads, D=64.
Sharding: data-parallel over batch dim B=8 -> 1 batch row per NeuronCore.
Each core processes 64 windows; per window a full MHA block in bf16 operands
with fp32 PSUM accumulation:
  - xT loaded directly via DMA xbar transpose (no PE transposes),
  - q/k/v projections with bf16 weights (LDWEIGHTS at 1 cyc/col),
  - transposed scores sT = k q^T per head; softmax via
    exp(s)*exp(mask+pos): exp(mask) precomputed host-side, exp(pos) const,
    em = emask*epos on GPSIMD, pexp = exp(sT)*em on DVE (all bf16),
  - z_aug = [v; 1]^T @ pexp gives z rows + denominator row; heads are
    processed in groups of 4 so score matmuls pair across PE row-groups
    (concurrent K=64 matmuls) and two heads share each PSUM bank for the
    denominator broadcast / reciprocal / normalize ops ([64,512] DVE ops),
  - out = z @ Wp^T + bp, f32 out.
Windows are software-pipelined: projection chunks of window w+1 are
interleaved with the attention head groups of window w.
"""
import os
import numpy as np
import ml_dtypes

import concourse.bass as bass
import concourse.mybir as mybir
import concourse.tile as tile
from concourse import bacc
from concourse.bass_utils import run_bass_kernel_spmd

B, W, S, E = 8, 64, 256, 512
H, D = 8, 64
SCALE = D ** -0.5
NCORES = 8
F32 = mybir.dt.float32
BF16 = mybir.dt.bfloat16
BF16NP = ml_dtypes.bfloat16
AOp = mybir.AluOpType
AF = mybir.ActivationFunctionType


def _emit(nc, tc, ctx, n_w, d):
    """Emit the per-core program: n_w windows of MHA."""
    const = ctx.enter_context(tc.tile_pool(name="const", bufs=1))

    # --- one-time constants ---
    w_sb = {}
    for name in ("wq", "wk", "wv", "wp"):
        t = const.tile([128, 4, E], BF16, tag=name)
        nc.sync.dma_start(t[:], d[name].rearrange("(ic p) o -> p ic o", p=128))
        w_sb[name] = t

    bqc = const.tile([128, 4], F32)
    nc.sync.dma_start(bqc[:], d["bq"][:])
    bkc = const.tile([128, 4], F32)
    nc.sync.dma_start(bkc[:], d["bk"][:])
    bv_bc = const.tile([128, E], F32)
    nc.sync.dma_start(bv_bc[:], d["bv"][:])
    bp_bc = const.tile([128, E], F32)
    nc.sync.dma_start(bp_bc[:], d["bp"][:])

    # exp(pos_bias)^T per head: [128 (j%128), h, jc, i] bf16
    epos = const.tile([128, H, 2, S], BF16)
    nc.sync.dma_start(epos[:], d["pos"].rearrange("h (c p) j -> p h c j", p=128))

    sel2 = const.tile([2, 128], BF16)
    nc.sync.dma_start(sel2[:], d["sel2"][:])

    # --- pools for the per-window pipeline ---
    emsk_p = ctx.enter_context(tc.tile_pool(name="emsk", bufs=2))
    em_p = ctx.enter_context(tc.tile_pool(name="em", bufs=2))
    xt_p = ctx.enter_context(tc.tile_pool(name="xt", bufs=2))
    qkv_p = ctx.enter_context(tc.tile_pool(name="qkv", bufs=2))
    zt_p = ctx.enter_context(tc.tile_pool(name="zt", bufs=2))
    outs_p = ctx.enter_context(tc.tile_pool(name="outs", bufs=2))
    exp_p = ctx.enter_context(tc.tile_pool(name="exp", bufs=4))
    pexp_p = ctx.enter_context(tc.tile_pool(name="pexp", bufs=4))
    den_p = ctx.enter_context(tc.tile_pool(name="den", bufs=4))
    rec_p = ctx.enter_context(tc.tile_pool(name="rec", bufs=2))

    ps_pj = ctx.enter_context(tc.tile_pool(name="ps_pj", bufs=2, space="PSUM"))
    ps_sc = ctx.enter_context(tc.tile_pool(name="ps_sc", bufs=3, space="PSUM"))
    ps_z = ctx.enter_context(tc.tile_pool(name="ps_z", bufs=2, space="PSUM"))
    ps_db = ctx.enter_context(tc.tile_pool(name="ps_db", bufs=1, space="PSUM"))

    def phase_a(w):
        """Load + project window w (dense PE work), em precompute (gpsimd)."""
        # xT [e, s] via DMA xbar transpose: [128 (e%128), ec, s] bf16
        xT = xt_p.tile([128, 4, S], BF16, tag="xT", name=f"xT{w}")
        nc.sync.dma_start_transpose(xT[:], d["x"][w])
        # exp(mask)^T window: [128 (j%128), jc, i] bf16
        emsk = emsk_p.tile([128, 2, S], BF16, tag="mk", name=f"mk{w}")
        nc.sync.dma_start(emsk[:], d["mask"][w].rearrange("(c p) j -> p c j", p=128))

        # em = exp(mask)^T * exp(pos)^T per head (gpsimd, sbuf bf16);
        # emitted as interleavable chunks so gpsimd pexp ops of the previous
        # window aren't queued behind them
        em = em_p.tile([128, H, 2, S], BF16, tag="em", name=f"em{w}")

        def em_chunk(h):
            nc.gpsimd.tensor_tensor(em[:, h], emsk[:], epos[:, h], AOp.mult)

        qT = qkv_p.tile([128, 4, S], BF16, tag="qT", name=f"qT{w}")
        kT = qkv_p.tile([128, 4, S], BF16, tag="kT", name=f"kT{w}")
        vA = qkv_p.tile([128, 2, H, 65], BF16, tag="vA", name=f"vA{w}")
        nc.gpsimd.memset(vA[:, :, :, 64:65], 1.0)

        def qk_chunk(oc, wt, dst, bias):
            p = ps_pj.tile([128, S], F32, tag="pj", name=f"pp{w}_{wt}_{oc}")
            for ic in range(4):
                nc.tensor.matmul(p[:], w_sb[wt][:, ic, oc * 128:(oc + 1) * 128],
                                 xT[:, ic], start=(ic == 0), stop=(ic == 3))
            nc.scalar.activation(dst[:, oc], p[:], AF.Identity,
                                 bias=bias[:, oc:oc + 1])

        def v_chunk(sc):
            pv = ps_pj.tile([128, E], F32, tag="pj", name=f"pv{w}_{sc}")
            for ic in range(4):
                nc.tensor.matmul(pv[:], xT[:, ic, sc * 128:(sc + 1) * 128],
                                 w_sb["wv"][:, ic], start=(ic == 0), stop=(ic == 3))
            nc.vector.scalar_tensor_tensor(
                vA[:, sc, :, 0:64], pv[:].rearrange("p (h o) -> p h o", h=H),
                0.0, bv_bc[:].rearrange("p (h o) -> p h o", h=H),
                AOp.bypass, AOp.add)

        chunks = []
        for oc in range(4):
            chunks.append(lambda oc=oc: qk_chunk(oc, "wq", qT, bqc))
            chunks.append(lambda oc=oc: qk_chunk(oc, "wk", kT, bkc))
        chunks.append(lambda: v_chunk(0))
        chunks.append(lambda: v_chunk(1))
        for h in range(H):
            chunks.append(lambda h=h: em_chunk(h))
        return (qT, kT, vA, em), chunks

    def phase_b(w, qT, kT, vA, em):
        """Attention + output projection for window w, as interleavable atoms."""
        zT = zt_p.tile([128, 4, S], BF16, tag="zT", name=f"zT{w}")

        def head_group(g4):
            """Heads 4*g4 .. 4*g4+3: paired scores, shared-bank softmax,
            one K=2 denominator broadcast + one reciprocal per group."""
            hs = [4 * g4 + i for i in range(4)]
            oc0 = 2 * g4
            sT = {}
            pexp = {}
            st = {}

            def scores(hpair):
                # alternating row groups (prow 0 / 64) for PE concurrency
                for jc in range(2):
                    for h in hpair:
                        oc, prow = h // 2, (h % 2) * 64
                        if h not in sT:
                            sT[h] = ps_sc.tile([128, 2, S], F32, tag="sc",
                                               name=f"sT{w}_{h}")
                        nc.tensor.matmul(
                            sT[h][:, jc],
                            kT[prow:prow + 64, oc, jc * 128:(jc + 1) * 128],
                            qT[prow:prow + 64, oc], start=True, stop=True)

            def soft(h):
                e0 = exp_p.tile([128, 2, S], BF16, tag="e0", name=f"e0{w}_{h}")
                nc.scalar.activation(e0[:], sT[h][:], AF.Exp)
                pexp[h] = pexp_p.tile([128, 2, S], BF16, tag="px",
                                      name=f"px{w}_{h}")
                if h < 2:
                    nc.gpsimd.tensor_tensor(pexp[h][:], e0[:], em[:, h], AOp.mult)
                else:
                    nc.vector.tensor_tensor(pexp[h][:], e0[:], em[:, h], AOp.mult)

            def zmm(hpair2, zp):
                # two heads (same prow, adjacent oc) share one PSUM bank
                za = ps_z.tile([65, 2, S], F32, tag="zz", name=f"za{w}_{zp}")
                first = True
                for col, h in enumerate(hpair2):
                    for jc in range(2):
                        nc.tensor.matmul(za[:, col], vA[:, jc, h], pexp[h][:, jc],
                                         start=first, stop=(col == 1 and jc == 1),
                                         skip_group_check=True)
                        first = False
                return za

            def dencp():
                # each pair's denominator row into its own [1, 512] tile
                dena = den_p.tile([1, 2, S], BF16, tag="den", name=f"dena{w}_{g4}")
                nc.scalar.copy(dena[:], st["a"][64:65])
                denb = den_p.tile([1, 2, S], BF16, tag="den", name=f"denb{w}_{g4}")
                nc.scalar.copy(denb[:], st["b"][64:65])
                st["dens"] = (dena, denb)

            def dbrec():
                # broadcast both pairs' denominators into one PSUM bank
                # (rows 0-63 <- den_a, rows 64-127 <- den_b), one reciprocal
                dena, denb = st["dens"]
                db = ps_db.tile([128, 2, S], F32, tag="db", name=f"db{w}_{g4}")
                nc.tensor.matmul(db[0:64], sel2[0:1, 0:64],
                                 dena[:].rearrange("p c s -> p (c s)"),
                                 start=True, stop=True, skip_group_check=True)
                nc.tensor.matmul(db[64:128], sel2[0:1, 0:64],
                                 denb[:].rearrange("p c s -> p (c s)"),
                                 start=True, stop=True, skip_group_check=True)
                rec = rec_p.tile([128, 2, S], F32, tag="rec", name=f"rec{w}_{g4}")
                nc.vector.reciprocal_approx_fast(rec[:], db[:])
                st["rec"] = rec

            def norm(key, prow):
                nc.vector.tensor_tensor(zT[prow:prow + 64, oc0:oc0 + 2],
                                        st[key][0:64], st["rec"][prow:prow + 64],
                                        AOp.mult)

            pa = (hs[0], hs[2])
            pb = (hs[1], hs[3])
            atoms = [
                lambda: scores(hs[0:2]),
                lambda: soft(hs[0]),
                lambda: scores(hs[2:4]),
                lambda: soft(hs[1]),
                lambda: soft(hs[2]),
                lambda: soft(hs[3]),
                lambda: st.__setitem__("a", zmm(pa, f"a{g4}")),
                lambda: st.__setitem__("b", zmm(pb, f"b{g4}")),
                dencp,
                dbrec,
                lambda: norm("a", 0),
                lambda: norm("b", 64),
            ]
            return atoms

        def tail():
            # output projection [s, o] natural + bias, then store
            out_sb = outs_p.tile([128, 2, E], F32, tag="osb", name=f"osb{w}")
            for sc in range(2):
                po = ps_pj.tile([128, E], F32, tag="pj", name=f"po{w}_{sc}")
                for ec in range(4):
                    nc.tensor.matmul(po[:], zT[:, ec, sc * 128:(sc + 1) * 128],
                                     w_sb["wp"][:, ec], start=(ec == 0), stop=(ec == 3))
                nc.vector.scalar_tensor_tensor(out_sb[:, sc], po[:], 0.0, bp_bc[:],
                                               AOp.bypass, AOp.add)
            nc.sync.dma_start(d["out"][w].rearrange("(c p) e -> p c e", p=128), out_sb[:])

        atoms = head_group(0) + head_group(1)
        return atoms, tail

    prev = None
    for w in range(n_w):
        cur, chunks = phase_a(w)
        if prev is not None:
            # interleave: projection/em chunks of window w woven evenly
            # between the attention atoms of window w-1
            atoms, tail = phase_b(w - 1, *prev)
            seq = []
            ci = 0
            na, nch = len(atoms), len(chunks)
            for ai, afn in enumerate(atoms):
                seq.append(afn)
                target = ((ai + 1) * nch) // na
                while ci < target:
                    seq.append(chunks[ci]); ci += 1
            seq.extend(chunks[ci:])
            seq.append(tail)
            for fn in seq:
                fn()
        else:
            for fn in chunks:
                fn()
        prev = cur
    atoms, tail = phase_b(n_w - 1, *prev)
    for fn in atoms:
        fn()
    tail()


def _build(n_w):
    nc = bacc.Bacc("TRN2", target_bir_lowering=False, debug=False)
    d = {
        "x": nc.dram_tensor("x", [n_w, S, E], BF16, kind="ExternalInput"),
        "mask": nc.dram_tensor("mask", [n_w, S, S], BF16, kind="ExternalInput"),
        "pos": nc.dram_tensor("pos", [H, S, S], BF16, kind="ExternalInput"),
        "wq": nc.dram_tensor("wq", [E, E], BF16, kind="ExternalInput"),
        "wk": nc.dram_tensor("wk", [E, E], BF16, kind="ExternalInput"),
        "wv": nc.dram_tensor("wv", [E, E], BF16, kind="ExternalInput"),
        "wp": nc.dram_tensor("wp", [E, E], BF16, kind="ExternalInput"),
        "bq": nc.dram_tensor("bq", [128, 4], F32, kind="ExternalInput"),
        "bk": nc.dram_tensor("bk", [128, 4], F32, kind="ExternalInput"),
        "bv": nc.dram_tensor("bv", [128, E], F32, kind="ExternalInput"),
        "bp": nc.dram_tensor("bp", [128, E], F32, kind="ExternalInput"),
        "sel2": nc.dram_tensor("sel2", [2, 128], BF16, kind="ExternalInput"),
        "out": nc.dram_tensor("out", [n_w, S, E], F32, kind="ExternalOutput"),
    }
    from contextlib import ExitStack
    with tile.TileContext(nc) as tc, ExitStack() as ctx:
        _emit(nc, tc, ctx, n_w, d)
    nc.compile()
    return nc


_NC_CACHE = {}


def _get_nc(n_w):
    if n_w not in _NC_CACHE:
        _NC_CACHE[n_w] = _build(n_w)
    return _NC_CACHE[n_w]


def _host_prep(mask, Wq, bq, Wk, bk, Wv, bv, Wp, bp, pos_bias):
    """Shared (replicated) input tensors, host-side layout prep."""
    f = np.float32
    wq_t = np.ascontiguousarray((np.asarray(Wq, f).T * SCALE).astype(BF16NP))
    wk_t = np.ascontiguousarray(np.asarray(Wk, f).T.astype(BF16NP))
    wv_t = np.ascontiguousarray(np.asarray(Wv, f).T.astype(BF16NP))
    wp_t = np.ascontiguousarray(np.asarray(Wp, f).T.astype(BF16NP))
    bq_s = (bq * SCALE).astype(f)
    # bias tiles for qT/kT layout: [128 (o%128), oc, s] broadcast along s
    bq_t = np.ascontiguousarray(bq_s.reshape(4, 128).T)
    bk_t = np.ascontiguousarray(np.asarray(bk, f).reshape(4, 128).T)
    bv_bc = np.ascontiguousarray(np.broadcast_to(np.asarray(bv, f)[None, :], (128, E)))
    bp_bc = np.ascontiguousarray(np.broadcast_to(np.asarray(bp, f)[None, :], (128, E)))
    # exp of transposed mask / pos_bias for the partition-axis softmax layout
    emaskt = np.ascontiguousarray(
        np.exp(np.asarray(mask, f)[0, :, 0].transpose(0, 2, 1)).astype(BF16NP))
    sel2 = np.ascontiguousarray((np.arange(128)[None, :] // 64 == np.arange(2)[:, None]).astype(BF16NP))
    epost = np.ascontiguousarray(
        np.exp(np.asarray(pos_bias, f).transpose(0, 2, 1)).astype(BF16NP))
    return {
        "wq": wq_t, "wk": wk_t, "wv": wv_t, "wp": wp_t,
        "bq": bq_t, "bk": bk_t, "bv": bv_bc, "bp": bp_bc,
        "pos": epost, "_maskt": emaskt,
        "sel2": sel2,
    }


def _make_in_maps(x, mask, Wq, bq, Wk, bk, Wv, bv, Wp, bp, pos_bias, n_w, n_cores):
    x = np.asarray(x, np.float32).astype(BF16NP)
    shared = _host_prep(mask, Wq, bq, Wk, bk, Wv, bv, Wp, bp, pos_bias)
    maskt = shared.pop("_maskt")[:n_w]

    in_maps = []
    for c in range(n_cores):
        m = dict(shared)
        m["mask"] = maskt
        m["x"] = np.ascontiguousarray(x[c % B, :n_w])
        in_maps.append(m)
    return in_maps


def kernel(x, mask, Wq, bq, Wk, bk, Wv, bv, Wp, bp, pos_bias, _trace=False):
    n_w = int(os.environ.get("KERNEL_NW", W))
    n_cores = NCORES
    in_maps = _make_in_maps(x, mask, Wq, bq, Wk, bk, Wv, bv, Wp, bp, pos_bias,
                            n_w, n_cores)

    nc = _get_nc(n_w)
    res = run_bass_kernel_spmd(nc, in_maps, list(range(n_cores)), trace=_trace,
                               tmpdir=(os.environ.get("KERNEL_TRACE_DIR") if _trace else None))
    out = np.stack([res.results[c]["out"] for c in range(B)], axis=0)
    if _trace:
        kernel._last_exec_time_ns = res.exec_time_ns
        kernel._last_results = res
    return out


# revision 25
# speedup vs baseline: 1.1928x; 1.1928x over previous
"""Trainium2 Bass kernel for batched windowed multi-head attention.

Shapes: x (8, 64, 256, 512) f32, H=8 heads, D=64.
Sharding: data-parallel over batch dim B=8 -> 1 batch row per NeuronCore.
Each core processes 64 windows; per window a full MHA block in bf16 operands
with fp32 PSUM accumulation:
  - xT loaded directly via DMA xbar transpose (no PE transposes),
  - q/k/v projections with bf16 weights (LDWEIGHTS at 1 cyc/col),
  - transposed scores sT = k q^T per head; softmax via
    exp(s)*exp(mask+pos): exp(mask) precomputed host-side, exp(pos) const,
    em = emask*epos on GPSIMD, pexp = exp(sT)*em on DVE (all bf16),
  - z_aug = [v; 1]^T @ pexp gives z rows + denominator row; heads are
    processed in groups of 4 so score matmuls pair across PE row-groups
    (concurrent K=64 matmuls) and two heads share each PSUM bank for the
    denominator broadcast / reciprocal / normalize ops ([64,512] DVE ops),
  - out = z @ Wp^T + bp, f32 out.
Windows are software-pipelined: projection chunks of window w+1 are
interleaved with the attention head groups of window w.
"""
import os
import numpy as np
import ml_dtypes

import concourse.bass as bass
import concourse.mybir as mybir
import concourse.tile as tile
from concourse import bacc
from concourse.bass_utils import run_bass_kernel_spmd

B, W, S, E = 8, 64, 256, 512
H, D = 8, 64
SCALE = D ** -0.5
NCORES = 8
F32 = mybir.dt.float32
BF16 = mybir.dt.bfloat16
BF16NP = ml_dtypes.bfloat16
AOp = mybir.AluOpType
AF = mybir.ActivationFunctionType


def _emit(nc, tc, ctx, n_w, d):
    """Emit the per-core program: n_w windows of MHA."""
    const = ctx.enter_context(tc.tile_pool(name="const", bufs=1))

    # --- one-time constants ---
    w_sb = {}
    for name in ("wq", "wk", "wv", "wp"):
        t = const.tile([128, 4, E], BF16, tag=name)
        nc.sync.dma_start(t[:], d[name].rearrange("(ic p) o -> p ic o", p=128))
        w_sb[name] = t

    bqc = const.tile([128, 4], F32)
    nc.sync.dma_start(bqc[:], d["bq"][:])
    bkc = const.tile([128, 4], F32)
    nc.sync.dma_start(bkc[:], d["bk"][:])
    bv_bc = const.tile([128, E], F32)
    nc.sync.dma_start(bv_bc[:], d["bv"][:])
    bp_bc = const.tile([128, E], F32)
    nc.sync.dma_start(bp_bc[:], d["bp"][:])

    # exp(pos_bias)^T per head: [128 (j%128), h, jc, i] bf16
    epos = const.tile([128, H, 2, S], BF16)
    nc.sync.dma_start(epos[:], d["pos"].rearrange("h (c p) j -> p h c j", p=128))

    sel2 = const.tile([2, 128], BF16)
    nc.sync.dma_start(sel2[:], d["sel2"][:])

    # --- pools for the per-window pipeline ---
    emsk_p = ctx.enter_context(tc.tile_pool(name="emsk", bufs=2))
    em_p = ctx.enter_context(tc.tile_pool(name="em", bufs=2))
    xt_p = ctx.enter_context(tc.tile_pool(name="xt", bufs=2))
    qkv_p = ctx.enter_context(tc.tile_pool(name="qkv", bufs=2))
    zt_p = ctx.enter_context(tc.tile_pool(name="zt", bufs=2))
    outs_p = ctx.enter_context(tc.tile_pool(name="outs", bufs=2))
    exp_p = ctx.enter_context(tc.tile_pool(name="exp", bufs=4))
    pexp_p = ctx.enter_context(tc.tile_pool(name="pexp", bufs=4))
    den_p = ctx.enter_context(tc.tile_pool(name="den", bufs=4))
    rec_p = ctx.enter_context(tc.tile_pool(name="rec", bufs=2))

    ps_pj = ctx.enter_context(tc.tile_pool(name="ps_pj", bufs=2, space="PSUM"))
    ps_sc = ctx.enter_context(tc.tile_pool(name="ps_sc", bufs=3, space="PSUM"))
    ps_z = ctx.enter_context(tc.tile_pool(name="ps_z", bufs=2, space="PSUM"))
    ps_db = ctx.enter_context(tc.tile_pool(name="ps_db", bufs=1, space="PSUM"))

    def phase_a(w):
        """Load + project window w (dense PE work), em precompute (gpsimd)."""
        # xT [e, s] via DMA xbar transpose: [128 (e%128), ec, s] bf16
        xT = xt_p.tile([128, 4, S], BF16, tag="xT", name=f"xT{w}")
        nc.sync.dma_start_transpose(xT[:], d["x"][w])
        # exp(mask)^T window: [128 (j%128), jc, i] bf16
        emsk = emsk_p.tile([128, 2, S], BF16, tag="mk", name=f"mk{w}")
        nc.sync.dma_start(emsk[:], d["mask"][w].rearrange("(c p) j -> p c j", p=128))

        # em = exp(mask)^T * exp(pos)^T per head (gpsimd, sbuf bf16);
        # emitted as interleavable chunks so gpsimd pexp ops of the previous
        # window aren't queued behind them
        em = em_p.tile([128, H, 2, S], BF16, tag="em", name=f"em{w}")

        def em_chunk(h):
            nc.gpsimd.tensor_tensor(em[:, h], emsk[:], epos[:, h], AOp.mult)

        qT = qkv_p.tile([128, 4, S], BF16, tag="qT", name=f"qT{w}")
        kT = qkv_p.tile([128, 4, S], BF16, tag="kT", name=f"kT{w}")
        vA = qkv_p.tile([128, 2, H, 65], BF16, tag="vA", name=f"vA{w}")
        nc.gpsimd.memset(vA[:, :, :, 64:65], 1.0)

        def qk_chunk(oc, wt, dst, bias):
            p = ps_pj.tile([128, S], F32, tag="pj", name=f"pp{w}_{wt}_{oc}")
            for ic in range(4):
                nc.tensor.matmul(p[:], w_sb[wt][:, ic, oc * 128:(oc + 1) * 128],
                                 xT[:, ic], start=(ic == 0), stop=(ic == 3))
            if wt == "wq":
                nc.scalar.activation(dst[:, oc], p[:], AF.Identity,
                                     bias=bias[:, oc:oc + 1])
            else:
                nc.vector.tensor_scalar(dst[:, oc], p[:], bias[:, oc:oc + 1],
                                        None, AOp.add)

        def v_chunk(sc):
            pv = ps_pj.tile([128, E], F32, tag="pj", name=f"pv{w}_{sc}")
            for ic in range(4):
                nc.tensor.matmul(pv[:], xT[:, ic, sc * 128:(sc + 1) * 128],
                                 w_sb["wv"][:, ic], start=(ic == 0), stop=(ic == 3))
            nc.vector.scalar_tensor_tensor(
                vA[:, sc, :, 0:64], pv[:].rearrange("p (h o) -> p h o", h=H),
                0.0, bv_bc[:].rearrange("p (h o) -> p h o", h=H),
                AOp.bypass, AOp.add)

        chunks = []
        for oc in range(4):
            chunks.append(lambda oc=oc: qk_chunk(oc, "wq", qT, bqc))
            chunks.append(lambda oc=oc: qk_chunk(oc, "wk", kT, bkc))
        chunks.append(lambda: v_chunk(0))
        chunks.append(lambda: v_chunk(1))
        for h in range(H):
            chunks.append(lambda h=h: em_chunk(h))
        return (qT, kT, vA, em), chunks

    def phase_b(w, qT, kT, vA, em):
        """Attention + output projection for window w, as interleavable atoms."""
        zT = zt_p.tile([128, 4, S], BF16, tag="zT", name=f"zT{w}")

        def head_group(g4):
            """Heads 4*g4 .. 4*g4+3: paired scores, shared-bank softmax,
            one K=2 denominator broadcast + one reciprocal per group."""
            hs = [4 * g4 + i for i in range(4)]
            oc0 = 2 * g4
            sT = {}
            pexp = {}
            st = {}

            def scores(hpair):
                # alternating row groups (prow 0 / 64) for PE concurrency
                for jc in range(2):
                    for h in hpair:
                        oc, prow = h // 2, (h % 2) * 64
                        if h not in sT:
                            sT[h] = ps_sc.tile([128, 2, S], F32, tag="sc",
                                               name=f"sT{w}_{h}")
                        nc.tensor.matmul(
                            sT[h][:, jc],
                            kT[prow:prow + 64, oc, jc * 128:(jc + 1) * 128],
                            qT[prow:prow + 64, oc], start=True, stop=True)

            def soft(h):
                e0 = exp_p.tile([128, 2, S], BF16, tag="e0", name=f"e0{w}_{h}")
                nc.scalar.activation(e0[:], sT[h][:], AF.Exp)
                pexp[h] = pexp_p.tile([128, 2, S], BF16, tag="px",
                                      name=f"px{w}_{h}")
                nc.vector.tensor_tensor(pexp[h][:], e0[:], em[:, h], AOp.mult)

            def zmm(hpair2, zp):
                # two heads (same prow, adjacent oc) share one PSUM bank
                za = ps_z.tile([65, 2, S], F32, tag="zz", name=f"za{w}_{zp}")
                first = True
                for col, h in enumerate(hpair2):
                    for jc in range(2):
                        nc.tensor.matmul(za[:, col], vA[:, jc, h], pexp[h][:, jc],
                                         start=first, stop=(col == 1 and jc == 1),
                                         skip_group_check=True)
                        first = False
                return za

            def dencp():
                # each pair's denominator row into its own [1, 512] tile
                dena = den_p.tile([1, 2, S], BF16, tag="den", name=f"dena{w}_{g4}")
                nc.scalar.copy(dena[:], st["a"][64:65])
                denb = den_p.tile([1, 2, S], BF16, tag="den", name=f"denb{w}_{g4}")
                nc.scalar.copy(denb[:], st["b"][64:65])
                st["dens"] = (dena, denb)

            def dbrec():
                # broadcast both pairs' denominators into one PSUM bank
                # (rows 0-63 <- den_a, rows 64-127 <- den_b), one reciprocal
                dena, denb = st["dens"]
                db = ps_db.tile([128, 2, S], F32, tag="db", name=f"db{w}_{g4}")
                nc.tensor.matmul(db[0:64], sel2[0:1, 0:64],
                                 dena[:].rearrange("p c s -> p (c s)"),
                                 start=True, stop=True, skip_group_check=True)
                nc.tensor.matmul(db[64:128], sel2[0:1, 0:64],
                                 denb[:].rearrange("p c s -> p (c s)"),
                                 start=True, stop=True, skip_group_check=True)
                rec = rec_p.tile([128, 2, S], F32, tag="rec", name=f"rec{w}_{g4}")
                nc.vector.reciprocal_approx_fast(rec[:], db[:])
                st["rec"] = rec

            def norm(key, prow):
                nc.vector.tensor_tensor(zT[prow:prow + 64, oc0:oc0 + 2],
                                        st[key][0:64], st["rec"][prow:prow + 64],
                                        AOp.mult)

            pa = (hs[0], hs[2])
            pb = (hs[1], hs[3])
            atoms = [
                lambda: scores(hs[0:2]),
                lambda: soft(hs[0]),
                lambda: scores(hs[2:4]),
                lambda: soft(hs[1]),
                lambda: soft(hs[2]),
                lambda: soft(hs[3]),
                lambda: st.__setitem__("a", zmm(pa, f"a{g4}")),
                lambda: st.__setitem__("b", zmm(pb, f"b{g4}")),
                dencp,
                dbrec,
                lambda: norm("a", 0),
                lambda: norm("b", 64),
            ]
            return atoms

        def tail():
            # output projection [s, o] natural + bias, then store
            out_sb = outs_p.tile([128, 2, E], F32, tag="osb", name=f"osb{w}")
            for sc in range(2):
                po = ps_pj.tile([128, E], F32, tag="pj", name=f"po{w}_{sc}")
                for ec in range(4):
                    nc.tensor.matmul(po[:], zT[:, ec, sc * 128:(sc + 1) * 128],
                                     w_sb["wp"][:, ec], start=(ec == 0), stop=(ec == 3))
                nc.vector.scalar_tensor_tensor(out_sb[:, sc], po[:], 0.0, bp_bc[:],
                                               AOp.bypass, AOp.add)
            nc.sync.dma_start(d["out"][w].rearrange("(c p) e -> p c e", p=128), out_sb[:])

        atoms = head_group(0) + head_group(1)
        return atoms, tail

    prev = None
    for w in range(n_w):
        cur, chunks = phase_a(w)
        if prev is not None:
            # interleave: projection/em chunks of window w woven evenly
            # between the attention atoms of window w-1
            atoms, tail = phase_b(w - 1, *prev)
            seq = []
            ci = 0
            na, nch = len(atoms), len(chunks)
            for ai, afn in enumerate(atoms):
                seq.append(afn)
                target = ((ai + 1) * nch) // na
                while ci < target:
                    seq.append(chunks[ci]); ci += 1
            seq.extend(chunks[ci:])
            seq.append(tail)
            for fn in seq:
                fn()
        else:
            for fn in chunks:
                fn()
        prev = cur
    atoms, tail = phase_b(n_w - 1, *prev)
    for fn in atoms:
        fn()
    tail()


def _build(n_w):
    nc = bacc.Bacc("TRN2", target_bir_lowering=False, debug=False)
    d = {
        "x": nc.dram_tensor("x", [n_w, S, E], BF16, kind="ExternalInput"),
        "mask": nc.dram_tensor("mask", [n_w, S, S], BF16, kind="ExternalInput"),
        "pos": nc.dram_tensor("pos", [H, S, S], BF16, kind="ExternalInput"),
        "wq": nc.dram_tensor("wq", [E, E], BF16, kind="ExternalInput"),
        "wk": nc.dram_tensor("wk", [E, E], BF16, kind="ExternalInput"),
        "wv": nc.dram_tensor("wv", [E, E], BF16, kind="ExternalInput"),
        "wp": nc.dram_tensor("wp", [E, E], BF16, kind="ExternalInput"),
        "bq": nc.dram_tensor("bq", [128, 4], F32, kind="ExternalInput"),
        "bk": nc.dram_tensor("bk", [128, 4], F32, kind="ExternalInput"),
        "bv": nc.dram_tensor("bv", [128, E], F32, kind="ExternalInput"),
        "bp": nc.dram_tensor("bp", [128, E], F32, kind="ExternalInput"),
        "sel2": nc.dram_tensor("sel2", [2, 128], BF16, kind="ExternalInput"),
        "out": nc.dram_tensor("out", [n_w, S, E], F32, kind="ExternalOutput"),
    }
    from contextlib import ExitStack
    with tile.TileContext(nc) as tc, ExitStack() as ctx:
        _emit(nc, tc, ctx, n_w, d)
    nc.compile()
    return nc


_NC_CACHE = {}


def _get_nc(n_w):
    if n_w not in _NC_CACHE:
        _NC_CACHE[n_w] = _build(n_w)
    return _NC_CACHE[n_w]


def _host_prep(mask, Wq, bq, Wk, bk, Wv, bv, Wp, bp, pos_bias):
    """Shared (replicated) input tensors, host-side layout prep."""
    f = np.float32
    wq_t = np.ascontiguousarray((np.asarray(Wq, f).T * SCALE).astype(BF16NP))
    wk_t = np.ascontiguousarray(np.asarray(Wk, f).T.astype(BF16NP))
    wv_t = np.ascontiguousarray(np.asarray(Wv, f).T.astype(BF16NP))
    wp_t = np.ascontiguousarray(np.asarray(Wp, f).T.astype(BF16NP))
    bq_s = (bq * SCALE).astype(f)
    # bias tiles for qT/kT layout: [128 (o%128), oc, s] broadcast along s
    bq_t = np.ascontiguousarray(bq_s.reshape(4, 128).T)
    bk_t = np.ascontiguousarray(np.asarray(bk, f).reshape(4, 128).T)
    bv_bc = np.ascontiguousarray(np.broadcast_to(np.asarray(bv, f)[None, :], (128, E)))
    bp_bc = np.ascontiguousarray(np.broadcast_to(np.asarray(bp, f)[None, :], (128, E)))
    # exp of transposed mask / pos_bias for the partition-axis softmax layout
    emaskt = np.ascontiguousarray(
        np.exp(np.asarray(mask, f)[0, :, 0].transpose(0, 2, 1)).astype(BF16NP))
    sel2 = np.ascontiguousarray((np.arange(128)[None, :] // 64 == np.arange(2)[:, None]).astype(BF16NP))
    epost = np.ascontiguousarray(
        np.exp(np.asarray(pos_bias, f).transpose(0, 2, 1)).astype(BF16NP))
    return {
        "wq": wq_t, "wk": wk_t, "wv": wv_t, "wp": wp_t,
        "bq": bq_t, "bk": bk_t, "bv": bv_bc, "bp": bp_bc,
        "pos": epost, "_maskt": emaskt,
        "sel2": sel2,
    }


def _make_in_maps(x, mask, Wq, bq, Wk, bk, Wv, bv, Wp, bp, pos_bias, n_w, n_cores):
    x = np.asarray(x, np.float32).astype(BF16NP)
    shared = _host_prep(mask, Wq, bq, Wk, bk, Wv, bv, Wp, bp, pos_bias)
    maskt = shared.pop("_maskt")[:n_w]

    in_maps = []
    for c in range(n_cores):
        m = dict(shared)
        m["mask"] = maskt
        m["x"] = np.ascontiguousarray(x[c % B, :n_w])
        in_maps.append(m)
    return in_maps


def kernel(x, mask, Wq, bq, Wk, bk, Wv, bv, Wp, bp, pos_bias, _trace=False):
    n_w = int(os.environ.get("KERNEL_NW", W))
    n_cores = NCORES
    in_maps = _make_in_maps(x, mask, Wq, bq, Wk, bk, Wv, bv, Wp, bp, pos_bias,
                            n_w, n_cores)

    nc = _get_nc(n_w)
    res = run_bass_kernel_spmd(nc, in_maps, list(range(n_cores)), trace=_trace,
                               tmpdir=(os.environ.get("KERNEL_TRACE_DIR") if _trace else None))
    out = np.stack([res.results[c]["out"] for c in range(B)], axis=0)
    if _trace:
        kernel._last_exec_time_ns = res.exec_time_ns
        kernel._last_results = res
    return out
